# revision 9
# baseline (speedup 1.0000x reference)
"""BitLinear (ternary weight + per-token int8 absmax activation) on 8 trn2 cores.

y = (round(x/s) clipped) * s  @  (alpha * clip(round(W/alpha),-1,1)).T  + bias
  with s = max(absmax(x, -1), eps)/127 per token, alpha = max(mean|W|, eps).

v2 strategy: data-parallel over tokens (4096 tokens/core), NO collectives.
Every core loads the full f32 W twice from HBM (streaming): pass 1 reduces
|W| partials for alpha (pairwise trees keep the sum within ~1 ulp so the
ternary rounding boundary, which sits ~4e-7 relative from the nearest
weight, is decided exactly like the f64 reference); pass 2 re-streams W,
ternarizes (magic-number RNE round) and DMA-transposes chunk-by-chunk
straight into the bf16 W^T tile, so the PE can start on the first dout
slice ~60us in, with no AllReduce/AllGather on the critical path.

Activations ride fp16 from the host (absmax/quant on-device; fp16 x only
perturbs the reference's int8 grid choice by ~0.3% of max|y|), y is stored
fp16 and upcast on the host (+0.05%).  Quantized activations (bf16
integers) x ternary weights (bf16) accumulate exactly in fp32 PSUM.

Weight-phase numerics stay f32 end-to-end: feeding W as fp16 flips ~100
ternary decisions and costs 2.6% error -- over the 2% budget.  Total
measured error of this scheme is ~0.3%.
"""

import numpy as np
from contextlib import ExitStack

import concourse.bass as bass
from concourse import bacc
import concourse.mybir as mybir
import concourse.tile as tile
from concourse.bass import ts
from concourse.bass_utils import run_bass_kernel_spmd
from concourse.masks import make_identity

P = 128
D_IN = 2048
D_OUT = 2048
KC = D_IN // P          # 16 contraction chunks
WC = D_OUT // P         # 16 weight row chunks
NFREE = 512             # matmul free dim (one PSUM bank of f32)
NT = D_OUT // NFREE     # 4 n-chunks
MAGIC = 12582912.0      # 1.5 * 2**23 : fp32 RNE rounding offset
EPS = 1e-5
CLAMP = float(np.nextafter(np.float32(1.5), np.float32(0.0)))  # largest f32 < 1.5
N_CORES = 8
ST = 2                  # token tiles per supertile

F32 = mybir.dt.float32
FP16 = mybir.dt.float16
BF16 = mybir.dt.bfloat16
Copy = mybir.ActivationFunctionType.Copy
Alu = mybir.AluOpType
AX = mybir.AxisListType


def _build(T: int, repeat: int = 1) -> bass.Bass:
    """Build the per-core program for T tokens (repeat>1: perf timing only)."""
    st = ST if T % (P * ST) == 0 else 1
    MS = T // (P * st)  # supertiles
    nc = bacc.Bacc(None, target_bir_lowering=False)

    x_d = nc.dram_tensor("x", [T, D_IN], FP16, kind="ExternalInput")
    w_d = nc.dram_tensor("w", [D_OUT, D_IN], F32, kind="ExternalInput")
    b_d = nc.dram_tensor("b", [D_OUT], F32, kind="ExternalInput")
    y_d = nc.dram_tensor("y", [T, D_OUT], FP16, kind="ExternalOutput")
    x_v = x_d.rearrange("(s a p) d -> s p a d", p=P, a=st)
    y_v = y_d.rearrange("(s a p) d -> s p a d", p=P, a=st)

    with tile.TileContext(nc) as tc, ExitStack() as ctx:
      const = ctx.enter_context(tc.tile_pool(name="const", bufs=1))
      wload = ctx.enter_context(tc.tile_pool(name="wload", bufs=2))
      wf32 = ctx.enter_context(tc.tile_pool(name="wf32", bufs=2))
      wtern = ctx.enter_context(tc.tile_pool(name="wtern", bufs=2))
      xin = ctx.enter_context(tc.tile_pool(name="xin", bufs=2))
      qmag = ctx.enter_context(tc.tile_pool(name="qmag", bufs=2))
      xq = ctx.enter_context(tc.tile_pool(name="xq", bufs=2))
      xt = ctx.enter_context(tc.tile_pool(name="xt", bufs=2))
      scl = ctx.enter_context(tc.tile_pool(name="scl", bufs=4))
      yout = ctx.enter_context(tc.tile_pool(name="yout", bufs=2))
      psum = ctx.enter_context(tc.tile_pool(name="psum", bufs=2, space="PSUM"))
      dram = ctx.enter_context(tc.tile_pool(name="dram", bufs=1, space="DRAM"))
      for _rep in range(repeat):
        wT = const.tile([P, KC, D_OUT], BF16)           # full ternary W^T
        bias_f32 = wf32.tile([P, D_OUT], F32, tag="wf")  # transient staging
        bias_bc = const.tile([P, D_OUT], FP16)
        ident = const.tile([P, P], F32)
        partial = const.tile([P, WC], F32)
        wsum = const.tile([P, 1], F32)
        alpha_sb = const.tile([P, 1], F32)
        inv_alpha = const.tile([P, 1], F32)
        alpha127 = const.tile([P, 1], F32)

        nc.scalar.dma_start(out=bias_f32[:], in_=b_d[None, :].to_broadcast((P, D_OUT)))
        nc.gpsimd.tensor_scalar(bias_bc[:], bias_f32[:], 0.0, None, op0=Alu.add)
        make_identity(nc, ident[:])

        # ---- phase W-A: alpha = max(mean|W|, eps), local full reduce -----
        # pairwise trees end-to-end: 128-elem groups -> 16 -> tree(16) per
        # partition, PE transpose, tree(128) -> ~1 ulp of the f64 sum.
        for c in range(WC):
            wc = wload.tile([P, D_IN], F32, tag="wa")
            nc.scalar.dma_start(out=wc[:], in_=w_d[ts(c, P), :])
            s1 = scl.tile([P, KC], F32, tag="s1")
            nc.vector.tensor_reduce(
                s1[:], wc.rearrange("p (a b) -> p a b", a=KC), axis=AX.X,
                op=Alu.add, apply_absolute_value=True,
            )
            nc.vector.tensor_reduce(
                partial[:, c : c + 1], s1[:], axis=AX.X, op=Alu.add
            )
        width = WC // 2
        while width >= 1:
            nc.vector.tensor_tensor(
                partial[:, 0:width], partial[:, 0:width],
                partial[:, width : 2 * width], op=Alu.add,
            )
            width //= 2
        nc.scalar.copy(wsum[:], partial[:, 0:1])
        # 128 per-partition totals -> one row (exact PE transpose), then a
        # pairwise tree of 7 adds.
        ps_t = psum.tile([1, P], F32, tag="ps")
        nc.tensor.transpose(ps_t[:], wsum[:], ident[:])
        row = const.tile([1, P], F32)
        nc.scalar.copy(row[:], ps_t[:])
        width = P // 2
        while width >= 1:
            nc.vector.tensor_tensor(
                row[0:1, 0:width], row[0:1, 0:width],
                row[0:1, width : 2 * width], op=Alu.add,
            )
            width //= 2
        al_sc = const.tile([1, 1], F32)
        nc.vector.tensor_scalar(
            al_sc[:], row[0:1, 0:1], 1.0 / (D_IN * D_OUT), EPS,
            op0=Alu.mult, op1=Alu.max,
        )
        # broadcast alpha to all partitions through a DRAM bounce
        al_d = dram.tile([1, 1], F32, name="al_d")
        nc.sync.dma_start(out=al_d[:], in_=al_sc[:])
        nc.scalar.dma_start(out=alpha_sb[:], in_=al_d[:].to_broadcast((P, 1)))
        nc.vector.reciprocal(inv_alpha[:], alpha_sb[:])
        nc.scalar.mul(alpha127[:], alpha_sb[:], 1.0 / 127.0)

        # ---- phase W-B: re-stream W, ternarize, transpose into W^T -------
        for c in range(WC):
            wc2 = wload.tile([P, D_IN], F32, tag="wb")
            nc.sync.dma_start(out=wc2[:], in_=w_d[ts(c, P), :])
            wf = wf32.tile([P, D_IN], F32, tag="wf")
            nc.scalar.activation(wf[:], wc2[:], Copy, scale=inv_alpha[:])
            # clamp to (-1.5, 1.5) so round gives {-1,0,1} (== clip(round,-1,1))
            nc.vector.tensor_scalar(
                wf[:], wf[:], CLAMP, -CLAMP, op0=Alu.min, op1=Alu.max
            )
            wt = wtern.tile([P, D_IN], BF16, tag="wt")
            nc.gpsimd.tensor_scalar(
                wt[:], wf[:], MAGIC, MAGIC, op0=Alu.add, op1=Alu.subtract
            )
            nc.scalar.dma_start_transpose(wT[:, :, ts(c, P)], wt[:])

        # ---- main token loop: supertiles of st*128 tokens ---------------
        for m in range(MS):
            x_t = xin.tile([P, st, D_IN], FP16, tag="x")
            nc.sync.dma_start(out=x_t[:], in_=x_v[m])

            absmax = scl.tile([P, st], F32, tag="absmax")
            m1 = scl.tile([P, st], F32, tag="m1")
            r = scl.tile([P, st], F32, tag="r")
            inv127 = scl.tile([P, st], F32, tag="inv127")
            c_vec = scl.tile([P, st], F32, tag="c_vec")

            nc.vector.tensor_reduce(
                absmax[:], x_t[:], axis=AX.X, op=Alu.max, apply_absolute_value=True
            )
            nc.vector.tensor_scalar(m1[:], absmax[:], EPS, None, op0=Alu.max)
            nc.vector.reciprocal(r[:], m1[:])
            nc.scalar.mul(inv127[:], r[:], 127.0)
            nc.scalar.mul(c_vec[:], m1[:], alpha127[:])

            # q = round(x * 127/m1) as bf16 integers (magic-number RNE)
            q_t = xq.tile([P, st, D_IN], BF16, tag="q")
            for a in range(st):
                qm = qmag.tile([P, D_IN], F32, tag="qm")
                nc.scalar.activation(
                    qm[:], x_t[:, a, :], Copy, bias=MAGIC,
                    scale=inv127[:, a : a + 1],
                )
                nc.vector.tensor_scalar(
                    q_t[:, a, :], qm[:], MAGIC, None, op0=Alu.subtract
                )

            # transpose to [i, t] layout for the matmul (ACT HWDGE ring)
            xT_t = xt.tile([P, st * KC, P], BF16, tag="xT")
            nc.scalar.dma_start_transpose(
                xT_t[:], q_t.rearrange("p a d -> p (a d)"))

            y_t = yout.tile([P, st, D_OUT], FP16, tag="y")
            for a in range(st):
                ps = psum.tile([P, NT, NFREE], F32, tag="ps", name="ps")
                if m < 2:
                    # early supertiles: n-outer so each n group only needs
                    # its four W^T chunks -- PE starts before the whole
                    # ternarize pass lands and tracks its trickle
                    for n in range(NT):
                        for k in range(KC):
                            nc.tensor.matmul(
                                ps[:, n, :],
                                xT_t[:, a * KC + k, :],
                                wT[:, k, ts(n, NFREE)],
                                start=(k == 0),
                                stop=(k == KC - 1),
                            )
                else:
                    for k in range(KC):
                        for n in range(NT):
                            nc.tensor.matmul(
                                ps[:, n, :],
                                xT_t[:, a * KC + k, :],
                                wT[:, k, ts(n, NFREE)],
                                start=(k == 0),
                                stop=(k == KC - 1),
                            )
                ps_flat = ps.rearrange("p a b -> p (a b)")
                nc.scalar.activation(
                    y_t[:, a, :], ps_flat, Copy, scale=c_vec[:, a : a + 1]
                )
            nc.vector.tensor_tensor(
                y_t[:], y_t[:],
                bias_bc[:, None, :].to_broadcast((P, st, D_OUT)), op=Alu.add,
            )
            nc.sync.dma_start(out=y_v[m], in_=y_t[:])

    nc.compile()
    return nc


_PROG_CACHE: dict[tuple, bass.Bass] = {}


def _get_prog(T: int, repeat: int = 1) -> bass.Bass:
    key = (T, repeat)
    if key not in _PROG_CACHE:
        _PROG_CACHE[key] = _build(T, repeat)
    return _PROG_CACHE[key]


def _make_in_maps(xf: np.ndarray, w: np.ndarray, b: np.ndarray, T: int):
    xh = xf.astype(np.float16)
    return [
        {
            "x": np.ascontiguousarray(xh[c * T : (c + 1) * T]),
            "w": w,
            "b": b,
        }
        for c in range(N_CORES)
    ]


def kernel(x: np.ndarray, weight: np.ndarray, bias: np.ndarray) -> np.ndarray:
    orig_shape = x.shape
    xf = np.ascontiguousarray(x.reshape(-1, D_IN).astype(np.float32, copy=False))
    n_tok = xf.shape[0]
    assert n_tok % N_CORES == 0
    T = n_tok // N_CORES
    w = np.ascontiguousarray(weight.astype(np.float32, copy=False))
    b = np.ascontiguousarray(bias.astype(np.float32, copy=False))

    nc = _get_prog(T)
    in_maps = _make_in_maps(xf, w, b, T)
    res = run_bass_kernel_spmd(nc, in_maps, core_ids=list(range(N_CORES)))
    y = np.concatenate([r["y"] for r in res.results], axis=0)
    return y.reshape(orig_shape[:-1] + (D_OUT,)).astype(np.float32)


# revision 18
# speedup vs baseline: 1.2360x; 1.2360x over previous
"""BitLinear (ternary weight + per-token int8 absmax activation) on 8 trn2 cores.

y = (round(x/s) clipped) * s  @  (alpha * clip(round(W/alpha),-1,1)).T  + bias
  with s = max(absmax(x, -1), eps)/127 per token, alpha = max(mean|W|, eps).

v2 strategy: data-parallel over tokens (4096 tokens/core), NO collectives.
Every core loads the full f32 W twice from HBM (streaming): pass 1 reduces
|W| partials for alpha (pairwise trees keep the sum within ~1 ulp so the
ternary rounding boundary, which sits ~4e-7 relative from the nearest
weight, is decided exactly like the f64 reference); pass 2 re-streams W,
ternarizes (magic-number RNE round) and DMA-transposes chunk-by-chunk
straight into the bf16 W^T tile, so the PE can start on the first dout
slice ~60us in, with no AllReduce/AllGather on the critical path.

Activations ride fp16 from the host (absmax/quant on-device; fp16 x only
perturbs the reference's int8 grid choice by ~0.3% of max|y|), y is stored
fp16 and upcast on the host (+0.05%).  Quantized activations (bf16
integers) x ternary weights (bf16) accumulate exactly in fp32 PSUM.

Weight-phase numerics stay f32 end-to-end: feeding W as fp16 flips ~100
ternary decisions and costs 2.6% error -- over the 2% budget.  Total
measured error of this scheme is ~0.3%.
"""

import numpy as np
from contextlib import ExitStack

import concourse.bass as bass
from concourse import bacc
import concourse.mybir as mybir
import concourse.tile as tile
from concourse.bass import ts
from concourse.bass_utils import run_bass_kernel_spmd
from concourse.masks import make_identity

P = 128
D_IN = 2048
D_OUT = 2048
KC = D_IN // P          # 16 contraction chunks
WC = D_OUT // P         # 16 weight row chunks
NFREE = 512             # matmul free dim (one PSUM bank of f32)
NT = D_OUT // NFREE     # 4 n-chunks
MAGIC = 12582912.0      # 1.5 * 2**23 : fp32 RNE rounding offset
EPS = 1e-5
CLAMP = float(np.nextafter(np.float32(1.5), np.float32(0.0)))  # largest f32 < 1.5
N_CORES = 8
ST = 2                  # token tiles per supertile

X_FP16 = True           # feed x as fp16 (halves x HBM read)
Y_FP16 = True           # store y as fp16 (halves y HBM write)
WTL_COPY = False        # transpose to contiguous tile + copy into W^T slice
SKIP_WPHASE = False     # diagnostic: memset W^T, skip alpha/ternarize
SKIP_XPREP = False      # diagnostic: memset xT, skip x load/quant/transpose
WT4 = True              # W^T as [P, WC, KC, P]: contiguous transpose writes

F32 = mybir.dt.float32
FP16 = mybir.dt.float16
BF16 = mybir.dt.bfloat16
Copy = mybir.ActivationFunctionType.Copy
Alu = mybir.AluOpType
AX = mybir.AxisListType


def _build(T: int, repeat: int = 1) -> bass.Bass:
    """Build the per-core program for T tokens (repeat>1: perf timing only)."""
    st = ST if T % (P * ST) == 0 else 1
    MS = T // (P * st)  # supertiles
    nc = bacc.Bacc(None, target_bir_lowering=False)

    x_d = nc.dram_tensor("x", [T, D_IN], FP16 if X_FP16 else F32,
                         kind="ExternalInput")
    w_d = nc.dram_tensor("w", [D_OUT, D_IN], F32, kind="ExternalInput")
    b_d = nc.dram_tensor("b", [D_OUT], F32, kind="ExternalInput")
    y_d = nc.dram_tensor("y", [T, D_OUT], FP16 if Y_FP16 else F32,
                         kind="ExternalOutput")
    x_v = x_d.rearrange("(s a p) d -> s p a d", p=P, a=st)
    y_v = y_d.rearrange("(s a p) d -> s p a d", p=P, a=st)

    with tile.TileContext(nc) as tc, ExitStack() as ctx:
      const = ctx.enter_context(tc.tile_pool(name="const", bufs=1))
      wload = ctx.enter_context(tc.tile_pool(name="wload", bufs=2))
      wf32 = ctx.enter_context(tc.tile_pool(name="wf32", bufs=2))
      wtern = ctx.enter_context(tc.tile_pool(name="wtern", bufs=2))
      xin = ctx.enter_context(tc.tile_pool(name="xin", bufs=2))
      qmag = ctx.enter_context(tc.tile_pool(name="qmag", bufs=2))
      xq = ctx.enter_context(tc.tile_pool(name="xq", bufs=2))
      xt = ctx.enter_context(tc.tile_pool(name="xt", bufs=2))
      scl = ctx.enter_context(tc.tile_pool(name="scl", bufs=4))
      yout = ctx.enter_context(tc.tile_pool(name="yout", bufs=2))
      psum = ctx.enter_context(tc.tile_pool(name="psum", bufs=2, space="PSUM"))
      dram = ctx.enter_context(tc.tile_pool(name="dram", bufs=1, space="DRAM"))
      for _rep in range(repeat):
        if WT4:
            wT4 = const.tile([P, WC, KC, P], BF16)      # full ternary W^T
        else:
            wT = const.tile([P, KC, D_OUT], BF16)       # full ternary W^T
        bias_f32 = wf32.tile([P, D_OUT], F32, tag="wf")  # transient staging
        bias_bc = const.tile([P, D_OUT], FP16)
        ident = const.tile([P, P], F32)
        partial = const.tile([P, WC], F32)
        wsum = const.tile([P, 1], F32)
        alpha_sb = const.tile([P, 1], F32)
        inv_alpha = const.tile([P, 1], F32)
        alpha127 = const.tile([P, 1], F32)

        nc.scalar.dma_start(out=bias_f32[:], in_=b_d[None, :].to_broadcast((P, D_OUT)))
        nc.gpsimd.tensor_scalar(bias_bc[:], bias_f32[:], 0.0, None, op0=Alu.add)
        make_identity(nc, ident[:])

        if SKIP_WPHASE:
            # fill W^T via plain loads + ACT converting copies (diagnostic)
            for c in range(WC):
                wcf = wload.tile([P, D_IN], F32, tag="wa")
                nc.sync.dma_start(out=wcf[:], in_=w_d[ts(c, P), :])
                if WT4:
                    nc.scalar.copy(wT4[:, c, :, :].rearrange("p a b -> p (a b)"), wcf[:])
                else:
                    nc.scalar.copy(wT[:, c, :], wcf[:])
            nc.vector.memset(alpha_sb[:], 1.0)
            nc.vector.memset(inv_alpha[:], 1.0)
            nc.vector.memset(alpha127[:], 1.0)
        # ---- phase W-A: alpha = max(mean|W|, eps), local full reduce -----
        # pairwise trees end-to-end: 128-elem groups -> 16 -> tree(16) per
        # partition, PE transpose, tree(128) -> ~1 ulp of the f64 sum.
        for c in range(0 if SKIP_WPHASE else WC):
            wc = wload.tile([P, D_IN], F32, tag="wa")
            nc.scalar.dma_start(out=wc[:], in_=w_d[ts(c, P), :])
            s1 = scl.tile([P, KC], F32, tag="s1")
            nc.vector.tensor_reduce(
                s1[:], wc.rearrange("p (a b) -> p a b", a=KC), axis=AX.X,
                op=Alu.add, apply_absolute_value=True,
            )
            nc.vector.tensor_reduce(
                partial[:, c : c + 1], s1[:], axis=AX.X, op=Alu.add
            )
        if not SKIP_WPHASE:
          width = WC // 2
          while width >= 1:
            nc.vector.tensor_tensor(
                partial[:, 0:width], partial[:, 0:width],
                partial[:, width : 2 * width], op=Alu.add,
            )
            width //= 2
          nc.scalar.copy(wsum[:], partial[:, 0:1])
          # 128 per-partition totals -> one row (exact PE transpose), then a
          # pairwise tree of 7 adds.
          ps_t = psum.tile([1, P], F32, tag="ps")
          nc.tensor.transpose(ps_t[:], wsum[:], ident[:])
          row = const.tile([1, P], F32)
          nc.scalar.copy(row[:], ps_t[:])
          width = P // 2
          while width >= 1:
            nc.vector.tensor_tensor(
                row[0:1, 0:width], row[0:1, 0:width],
                row[0:1, width : 2 * width], op=Alu.add,
            )
            width //= 2
          al_sc = const.tile([1, 1], F32)
          nc.vector.tensor_scalar(
            al_sc[:], row[0:1, 0:1], 1.0 / (D_IN * D_OUT), EPS,
            op0=Alu.mult, op1=Alu.max,
          )
          # broadcast alpha to all partitions through a DRAM bounce
          al_d = dram.tile([1, 1], F32, name="al_d")
          nc.sync.dma_start(out=al_d[:], in_=al_sc[:])
          nc.scalar.dma_start(out=alpha_sb[:], in_=al_d[:].to_broadcast((P, 1)))
          nc.vector.reciprocal(inv_alpha[:], alpha_sb[:])
          nc.scalar.mul(alpha127[:], alpha_sb[:], 1.0 / 127.0)

        # ---- phase W-B: re-stream W, ternarize, transpose into W^T -------
        for c in range(0 if SKIP_WPHASE else WC):
            wc2 = wload.tile([P, D_IN], F32, tag="wb")
            nc.sync.dma_start(out=wc2[:], in_=w_d[ts(c, P), :])
            wf = wf32.tile([P, D_IN], F32, tag="wf")
            nc.scalar.activation(wf[:], wc2[:], Copy, scale=inv_alpha[:])
            # clamp to (-1.5, 1.5) so round gives {-1,0,1} (== clip(round,-1,1))
            nc.vector.tensor_scalar(
                wf[:], wf[:], CLAMP, -CLAMP, op0=Alu.min, op1=Alu.max
            )
            wt = wtern.tile([P, D_IN], BF16, tag="wt")
            nc.gpsimd.tensor_scalar(
                wt[:], wf[:], MAGIC, MAGIC, op0=Alu.add, op1=Alu.subtract
            )
            if WT4:
                nc.scalar.dma_start_transpose(wT4[:, c, :, :], wt[:])
            elif WTL_COPY:
                wtl = wtern.tile([P, KC, P], BF16, tag="wtl", bufs=1)
                nc.scalar.dma_start_transpose(wtl[:], wt[:])
                nc.scalar.dma_start(out=wT[:, :, ts(c, P)], in_=wtl[:])
            else:
                nc.scalar.dma_start_transpose(wT[:, :, ts(c, P)], wt[:])

        # ---- main token loop: supertiles of st*128 tokens ---------------
        for m in range(MS):
            c_vec = scl.tile([P, st], F32, tag="c_vec")
            if SKIP_XPREP:
                nc.vector.memset(c_vec[:], 1.0)
                x_t = xin.tile([P, st, D_IN], FP16 if X_FP16 else F32, tag="x")
                nc.sync.dma_start(out=x_t[:], in_=x_v[m])
                xT_t = xt.tile([P, st * KC, P], BF16, tag="xT")
                nc.scalar.copy(xT_t.rearrange("p a b -> p (a b)"),
                               x_t.rearrange("p a d -> p (a d)"))
            else:
                x_t = xin.tile([P, st, D_IN], FP16 if X_FP16 else F32, tag="x")
                nc.sync.dma_start(out=x_t[:], in_=x_v[m])

                absmax = scl.tile([P, st], F32, tag="absmax")
                m1 = scl.tile([P, st], F32, tag="m1")
                r = scl.tile([P, st], F32, tag="r")
                inv127 = scl.tile([P, st], F32, tag="inv127")

                nc.vector.tensor_reduce(
                    absmax[:], x_t[:], axis=AX.X, op=Alu.max, apply_absolute_value=True
                )
                nc.vector.tensor_scalar(m1[:], absmax[:], EPS, None, op0=Alu.max)
                nc.vector.reciprocal(r[:], m1[:])
                nc.scalar.mul(inv127[:], r[:], 127.0)
                nc.scalar.mul(c_vec[:], m1[:], alpha127[:])

                # q = round(x * 127/m1) as bf16 integers (magic-number RNE)
                q_t = xq.tile([P, st, D_IN], BF16, tag="q")
                for a in range(st):
                    qm = qmag.tile([P, D_IN], F32, tag="qm")
                    nc.scalar.activation(
                        qm[:], x_t[:, a, :], Copy, bias=MAGIC,
                        scale=inv127[:, a : a + 1],
                    )
                    nc.vector.tensor_scalar(
                        q_t[:, a, :], qm[:], MAGIC, None, op0=Alu.subtract
                    )

                # transpose to [i, t] layout for the matmul (ACT HWDGE ring)
                xT_t = xt.tile([P, st * KC, P], BF16, tag="xT")
                nc.scalar.dma_start_transpose(
                    xT_t[:], q_t.rearrange("p a d -> p (a d)"))

            y_t = yout.tile([P, st, D_OUT], FP16 if Y_FP16 else F32, tag="y")
            for a in range(st):
                ps = psum.tile([P, NT, NFREE], F32, tag="ps", name="ps")
                def rhs_ap(k, n):
                    if WT4:
                        return wT4[:, 4 * n : 4 * n + 4, k, :]
                    return wT[:, k, ts(n, NFREE)]
                if m < 2:
                    # early supertiles: n-outer so each n group only needs
                    # its four W^T chunks -- PE starts before the whole
                    # ternarize pass lands and tracks its trickle
                    for n in range(NT):
                        for k in range(KC):
                            nc.tensor.matmul(
                                ps[:, n, :],
                                xT_t[:, a * KC + k, :],
                                rhs_ap(k, n),
                                start=(k == 0),
                                stop=(k == KC - 1),
                            )
                else:
                    for k in range(KC):
                        for n in range(NT):
                            nc.tensor.matmul(
                                ps[:, n, :],
                                xT_t[:, a * KC + k, :],
                                rhs_ap(k, n),
                                start=(k == 0),
                                stop=(k == KC - 1),
                            )
                ps_flat = ps.rearrange("p a b -> p (a b)")
                nc.scalar.activation(
                    y_t[:, a, :], ps_flat, Copy, scale=c_vec[:, a : a + 1]
                )
            nc.vector.tensor_tensor(
                y_t[:], y_t[:],
                bias_bc[:, None, :].to_broadcast((P, st, D_OUT)), op=Alu.add,
            )
            nc.sync.dma_start(out=y_v[m], in_=y_t[:])

    nc.compile()
    return nc


_PROG_CACHE: dict[tuple, bass.Bass] = {}


def _get_prog(T: int, repeat: int = 1) -> bass.Bass:
    key = (T, repeat)
    if key not in _PROG_CACHE:
        _PROG_CACHE[key] = _build(T, repeat)
    return _PROG_CACHE[key]


def _make_in_maps(xf: np.ndarray, w: np.ndarray, b: np.ndarray, T: int):
    xh = xf.astype(np.float16) if X_FP16 else xf
    return [
        {
            "x": np.ascontiguousarray(xh[c * T : (c + 1) * T]),
            "w": w,
            "b": b,
        }
        for c in range(N_CORES)
    ]


def kernel(x: np.ndarray, weight: np.ndarray, bias: np.ndarray) -> np.ndarray:
    orig_shape = x.shape
    xf = np.ascontiguousarray(x.reshape(-1, D_IN).astype(np.float32, copy=False))
    n_tok = xf.shape[0]
    assert n_tok % N_CORES == 0
    T = n_tok // N_CORES
    w = np.ascontiguousarray(weight.astype(np.float32, copy=False))
    b = np.ascontiguousarray(bias.astype(np.float32, copy=False))

    nc = _get_prog(T)
    in_maps = _make_in_maps(xf, w, b, T)
    res = run_bass_kernel_spmd(nc, in_maps, core_ids=list(range(N_CORES)))
    y = np.concatenate([r["y"] for r in res.results], axis=0)
    return y.reshape(orig_shape[:-1] + (D_OUT,)).astype(np.float32)
